# revision 1
# baseline (speedup 1.0000x reference)
"""DeepSpeedMLP (pre-LN fp32 path) on 8 Trainium2 NeuronCores.

Sharding: data-parallel over tokens (8192 tokens -> 1024/core); every core
streams the full inter_w/output_w from HBM exactly once while holding the
transposed LN activations and the current I-block of gelu activations
SBUF-resident.  Matmuls run as float32r (fp22 mantissa, 1 PE pass) giving
~1e-4 relative error at bf16-class throughput.

Per-core pipeline:
  stage 1: res = input+residual+bias; LN; 128x128 PE transposes -> lnT
           (gamma/beta fused into the PSUM->SBUF copy); out is seeded
           with res (the final residual add).
  stage 2: for each 2048-wide I-block:
           GEMM1  psum[t,i] += lnT_k.T @ W1[k, iblock]   (lnT stationary)
           evict: psum -> f32 stage -> PE transpose -> gelu(x+b1) -> inter
           GEMM2  psum[t,h] += inter_i.T @ W2[iblock, h] (inter stationary)
           out[t,h] += psum  (DMA load/add/store; output_b added via a
           K=1 ones-matmul into the last block's psum accumulation)
"""
import sys
if '/opt/trn_rl_repo' not in sys.path:
    sys.path.insert(0, '/opt/trn_rl_repo')

import numpy as np
import concourse.bass as bass
import concourse.mybir as mybir
import concourse.tile as tile
from concourse import bacc
from concourse.bass_utils import run_bass_kernel_spmd

dt = mybir.dt
AF = mybir.ActivationFunctionType
ALU = mybir.AluOpType

N_CORES = 8
B, S, HIDDEN, INTER = 4, 2048, 4096, 16384
TOK = B * S
T = TOK // N_CORES       # tokens per core
IBLK = 2048              # I-block width
EPS = 1e-5


def _build_nc(H, I, T, IBLK):
    KS = H // 128     # H k-slabs
    TT = T // 128     # token tiles
    NB = I // IBLK    # I blocks
    IC = IBLK // 512  # 512-wide i-chunks per block (GEMM1 psum N)
    IS = IBLK // 128  # 128-row i-slabs per block (GEMM2 lhsT)
    HC = H // 512     # 512-wide h-chunks (GEMM2 psum N)
    SW = min(H, 2048)  # stage-1 strip width
    NS = H // SW

    nc = bacc.Bacc(None, target_bir_lowering=False)
    P = nc.declare_dram_parameter
    x_d = P("x", [T, H], dt.float32, isOutput=False)
    r_d = P("r", [T, H], dt.float32, isOutput=False)
    g_d = P("gamma_t", [128, KS], dt.float32, isOutput=False)
    be_d = P("beta_t", [128, KS], dt.float32, isOutput=False)
    w1_d = P("w1", [H, I], dt.float32r, isOutput=False)
    b1_d = P("b1_t", [128, I // 128], dt.float32, isOutput=False)
    w2_d = P("w2", [I, H], dt.float32r, isOutput=False)
    b2_d = P("b2", [1, H], dt.float32r, isOutput=False)
    ones_d = P("ones", [1, 128], dt.float32r, isOutput=False)
    id_d = P("ident", [128, 128], dt.float32, isOutput=False)
    o_d = P("out", [T, H], dt.float32, isOutput=True)

    with tile.TileContext(nc) as tc:
        with (
            tc.tile_pool(name="const", bufs=1) as constp,
            tc.tile_pool(name="lnT", bufs=1) as lnTp,
            tc.tile_pool(name="psum", bufs=8, space="PSUM") as psum,
        ):
            ident = constp.tile([128, 128], dt.float32)
            nc.sync.dma_start(out=ident[:], in_=id_d[:])
            g_sb = constp.tile([128, KS], dt.float32)
            nc.sync.dma_start(out=g_sb[:], in_=g_d[:])
            be_sb = constp.tile([128, KS], dt.float32)
            nc.sync.dma_start(out=be_sb[:], in_=be_d[:])
            b1_sb = constp.tile([128, I // 128], dt.float32)
            nc.sync.dma_start(out=b1_sb[:], in_=b1_d[:])
            ones = constp.tile([1, 128], dt.float32r)
            nc.sync.dma_start(out=ones[:], in_=ones_d[:])

            lnT = lnTp.tile([128, KS, T], dt.float32r)

            # ---- Stage 1: residual add + LN + transpose ----
            with (
                tc.tile_pool(name="s1in", bufs=2) as inp,
                tc.tile_pool(name="s1res", bufs=1) as resp,
                tc.tile_pool(name="s1st", bufs=2) as stp,
            ):
                for t in range(TT):
                    tr = slice(t * 128, (t + 1) * 128)
                    res = resp.tile([128, H], dt.float32, name="res")
                    for s in range(NS):
                        cs = slice(s * SW, (s + 1) * SW)
                        xs = inp.tile([128, SW], dt.float32, name="xt")
                        rs = inp.tile([128, SW], dt.float32, name="rt")
                        nc.sync.dma_start(out=xs[:], in_=x_d[tr, cs])
                        nc.sync.dma_start(out=rs[:], in_=r_d[tr, cs])
                        nc.vector.tensor_add(res[:, cs], xs[:], rs[:])
                    nc.sync.dma_start(out=o_d[tr, :], in_=res[:])

                    s1 = stp.tile([128, 1], dt.float32, name="s1")
                    nc.vector.tensor_reduce(s1[:], res[:], mybir.AxisListType.X, ALU.add)
                    s2 = stp.tile([128, 1], dt.float32, name="s2")
                    for s in range(NS):
                        cs = slice(s * SW, (s + 1) * SW)
                        junk = inp.tile([128, SW], dt.float32, name="xt")
                        s2p = stp.tile([128, 1], dt.float32, name="s2p")
                        nc.scalar.activation(junk[:], res[:, cs], AF.Square,
                                             accum_out=s2p[:])
                        if s == 0:
                            nc.vector.tensor_copy(s2[:], s2p[:])
                        else:
                            nc.vector.tensor_add(s2[:], s2[:], s2p[:])
                    mu = stp.tile([128, 1], dt.float32, name="mu")
                    nc.vector.tensor_scalar_mul(mu[:], s1[:], 1.0 / H)
                    mu2 = stp.tile([128, 1], dt.float32, name="mu2")
                    nc.vector.tensor_mul(mu2[:], mu[:], mu[:])
                    var = stp.tile([128, 1], dt.float32, name="var")
                    nc.vector.tensor_scalar(var[:], s2[:], 1.0 / H, float(EPS),
                                            ALU.mult, ALU.add)
                    nc.vector.tensor_sub(var[:], var[:], mu2[:])
                    sd = stp.tile([128, 1], dt.float32, name="sd")
                    nc.scalar.activation(sd[:], var[:], AF.Sqrt)
                    rstd = stp.tile([128, 1], dt.float32, name="rstd")
                    nc.vector.reciprocal(rstd[:], sd[:])
                    nmr = stp.tile([128, 1], dt.float32, name="nmr")
                    nc.vector.tensor_mul(nmr[:], mu[:], rstd[:])
                    nc.vector.tensor_scalar_mul(nmr[:], nmr[:], -1.0)

                    for s in range(NS):
                        cs = slice(s * SW, (s + 1) * SW)
                        lnp = inp.tile([128, SW], dt.float32, name="rt")
                        nc.scalar.activation(lnp[:], res[:, cs], AF.Identity,
                                             bias=nmr[:], scale=rstd[:])
                        for q in range(SW // 512):
                            pt = psum.tile([128, 512], dt.float32, name="ps")
                            for j in range(4):
                                nc.tensor.transpose(
                                    pt[:, j * 128:(j + 1) * 128],
                                    lnp[:, q * 512 + j * 128: q * 512 + (j + 1) * 128],
                                    ident[:])
                            for j in range(4):
                                k = (s * SW + q * 512) // 128 + j
                                nc.vector.tensor_scalar(
                                    lnT[:, k, t * 128:(t + 1) * 128],
                                    pt[:, j * 128:(j + 1) * 128],
                                    g_sb[:, k:k + 1], be_sb[:, k:k + 1],
                                    ALU.mult, ALU.add)

            # ---- Stage 2: per I-block GEMM1 -> gelu -> GEMM2 ----
            with (
                tc.tile_pool(name="interp", bufs=1) as interp,
                tc.tile_pool(name="wt", bufs=3) as wtp,
                tc.tile_pool(name="stage", bufs=3) as stgp,
                tc.tile_pool(name="b2sl", bufs=1) as b2p,
            ):
                inter = interp.tile([128, IS, T], dt.float32r)
                for b in range(NB):
                    for ic in range(IC):
                        pA = [psum.tile([128, 512], dt.float32, name="ps")
                              for _ in range(TT)]
                        for k in range(KS):
                            w1t = wtp.tile([128, 512], dt.float32r, name="wt")
                            nc.sync.dma_start(
                                out=w1t[:],
                                in_=w1_d[k * 128:(k + 1) * 128,
                                         b * IBLK + ic * 512: b * IBLK + (ic + 1) * 512])
                            for t in range(TT):
                                nc.tensor.matmul(
                                    pA[t][:], lnT[:, k, t * 128:(t + 1) * 128], w1t[:],
                                    start=(k == 0), stop=(k == KS - 1))
                        for t in range(TT):
                            s = stgp.tile([128, 512], dt.float32, name="stage")
                            nc.scalar.activation(s[:], pA[t][:], AF.Copy)
                            pT = psum.tile([128, 512], dt.float32, name="ps")
                            for j in range(4):
                                nc.tensor.transpose(
                                    pT[:, j * 128:(j + 1) * 128],
                                    s[:, j * 128:(j + 1) * 128], ident[:])
                            for j in range(4):
                                slab = ic * 4 + j
                                nc.scalar.activation(
                                    inter[:, slab, t * 128:(t + 1) * 128],
                                    pT[:, j * 128:(j + 1) * 128], AF.Gelu_apprx_tanh,
                                    bias=b1_sb[:, b * IS + slab: b * IS + slab + 1])
                    last = (b == NB - 1)
                    for h in range(HC):
                        if last:
                            b2s = b2p.tile([1, 512], dt.float32r, name="b2sl")
                            nc.sync.dma_start(out=b2s[:],
                                              in_=b2_d[:, h * 512:(h + 1) * 512])
                        pB = [psum.tile([128, 512], dt.float32, name="ps")
                              for _ in range(TT)]
                        for i in range(IS):
                            w2t = wtp.tile([128, 512], dt.float32r, name="wt")
                            nc.sync.dma_start(
                                out=w2t[:],
                                in_=w2_d[b * IBLK + i * 128: b * IBLK + (i + 1) * 128,
                                         h * 512:(h + 1) * 512])
                            for t in range(TT):
                                nc.tensor.matmul(
                                    pB[t][:], inter[:, i, t * 128:(t + 1) * 128], w2t[:],
                                    start=(i == 0),
                                    stop=(i == IS - 1) and not last)
                        if last:
                            for t in range(TT):
                                nc.tensor.matmul(pB[t][:], ones[:], b2s[:],
                                                 start=False, stop=True)
                        for t in range(TT):
                            tr = slice(t * 128, (t + 1) * 128)
                            hs = slice(h * 512, (h + 1) * 512)
                            s2t = stgp.tile([128, 512], dt.float32, name="stage")
                            nc.sync.dma_start(out=s2t[:], in_=o_d[tr, hs])
                            nc.vector.tensor_add(s2t[:], s2t[:], pB[t][:])
                            nc.sync.dma_start(out=o_d[tr, hs], in_=s2t[:])
    nc.compile()
    return nc


_NC_CACHE = None


def _get_nc():
    global _NC_CACHE
    if _NC_CACHE is None:
        _NC_CACHE = _build_nc(HIDDEN, INTER, T, IBLK)
    return _NC_CACHE


def kernel(input, residual, residual_norm, bias, attn_nw, attn_nb,
           inter_w, inter_b, output_w, output_b, **kwargs):
    H, I = HIDDEN, INTER
    KS = H // 128
    nc = _get_nc()

    x = np.ascontiguousarray(np.asarray(input, np.float32).reshape(TOK, H))
    r2 = np.asarray(residual, np.float32).reshape(TOK, H) + \
        np.asarray(bias, np.float32)[None, :]
    gamma_t = np.ascontiguousarray(np.asarray(attn_nw, np.float32).reshape(KS, 128).T)
    beta_t = np.ascontiguousarray(np.asarray(attn_nb, np.float32).reshape(KS, 128).T)
    b1_t = np.ascontiguousarray(np.asarray(inter_b, np.float32).reshape(I // 128, 128).T)
    b2 = np.ascontiguousarray(np.asarray(output_b, np.float32)[None, :])
    w1 = np.ascontiguousarray(np.asarray(inter_w, np.float32))
    w2 = np.ascontiguousarray(np.asarray(output_w, np.float32))
    ident = np.eye(128, dtype=np.float32)
    ones = np.ones((1, 128), np.float32)

    maps = []
    for c in range(N_CORES):
        sl = slice(c * T, (c + 1) * T)
        maps.append({
            'x': x[sl], 'r': np.ascontiguousarray(r2[sl]),
            'gamma_t': gamma_t, 'beta_t': beta_t,
            'w1': w1, 'b1_t': b1_t, 'w2': w2, 'b2': b2,
            'ones': ones, 'ident': ident,
        })
    res = run_bass_kernel_spmd(nc, maps, list(range(N_CORES)))
    out = np.concatenate([res.results[c]['out'] for c in range(N_CORES)], 0)
    return out.reshape(B, S, H).astype(np.float32)



# revision 14
# speedup vs baseline: 1.2577x; 1.2577x over previous
"""DeepSpeedMLP (pre-LN fp32 path) on 8 Trainium2 NeuronCores.

Sharding: data-parallel over tokens (8192 tokens -> 1024/core); every core
streams the full inter_w/output_w (bf16) from HBM once per 512-token chunk.

Per-core pipeline, per 512-token chunk (2 chunks):
  stage 1: res (precomputed x+residual+bias on host) -> LN stats; LN apply;
           128x128 PE transposes -> lnT[h, t] bf16 (gamma/beta fused into the
           PSUM->SBUF eviction).
  GEMM1 (weight-stationary): psum[i, 512t] += w1[hslab, islab].T @ lnT[hslab]
           accumulated over all 32 h-slabs; eviction = gelu(psum + b1) -> bf16
           interT[i, t] held SBUF-resident (no transposes needed).
  GEMM2 (token-stationary): psum[t, 512h] += interT[islab, ttile].T @
           w2[islab, hchunk] accumulated over all 128 i-slabs; output_b added
           via a K=1 ones-matmul; eviction = psum + res tile -> out (single
           store, no DRAM accumulation round-trip).
"""
import sys
if '/opt/trn_rl_repo' not in sys.path:
    sys.path.insert(0, '/opt/trn_rl_repo')

import numpy as np
import ml_dtypes
import concourse.bass as bass
import concourse.mybir as mybir
import concourse.tile as tile
from concourse import bacc
from concourse.bass_utils import run_bass_kernel_spmd

dt = mybir.dt
AF = mybir.ActivationFunctionType
ALU = mybir.AluOpType

N_CORES = 8
B, S, HIDDEN, INTER = 4, 2048, 4096, 16384
TOK = B * S
T = TOK // N_CORES       # tokens per core
TC = 512                 # tokens per chunk
EPS = 1e-5
FP8_GEMM2 = False        # fp8e4 DoubleRow second GEMM (2x PE rate)
W2_SCALE = 128.0         # fp8 storage scale for output_w (std 1/128 -> ~1)


def _build_nc(H, I, T, TC, gelu_af=AF.Gelu_apprx_tanh, fp8_g2=FP8_GEMM2):
    KS = H // 128            # h-slabs
    IS = I // 128            # i-slabs
    NCH = T // TC            # token chunks
    TT = TC // 128           # t-tiles per chunk
    HCW = min(512, H)        # h-chunk width (GEMM2 psum N)
    HCN = H // HCW
    IBW = min(512, I)        # i-block width (GEMM1 psum group)
    IBN = I // IBW
    IBS = IBW // 128         # i-slabs per i-block
    SW = min(H, 1024)        # stage-1 strip width
    NS = H // SW

    nc = bacc.Bacc(None, target_bir_lowering=False)
    P = nc.declare_dram_parameter
    res_d = P("res", [T, H], dt.float32, isOutput=False)
    sm_d = P("sm", [128, 2 * KS + IS], dt.float32, isOutput=False)
    w1_d = P("w1", [H, I], dt.bfloat16, isOutput=False)
    if fp8_g2:
        w2_d = P("w2", [I // 256, 128, 2, H], dt.float8e4, isOutput=False)
        i_dt = dt.float8e4
    else:
        w2_d = P("w2", [I, H], dt.bfloat16, isOutput=False)
        i_dt = dt.bfloat16
    b2_d = P("b2", [1, H], dt.bfloat16, isOutput=False)
    ones_d = P("ones", [1, 128], dt.bfloat16, isOutput=False)
    id_d = P("ident", [128, 128], dt.float32, isOutput=False)
    o_d = P("out", [T, H], dt.float32, isOutput=True)

    with tile.TileContext(nc) as tc:
        with (
            tc.tile_pool(name="const", bufs=1) as constp,
            tc.tile_pool(name="lnT", bufs=1) as lnTp,
            tc.tile_pool(name="interT", bufs=1) as interTp,
            tc.tile_pool(name="w1p", bufs=3) as w1p,
            tc.tile_pool(name="w2p", bufs=3) as w2p,
            tc.tile_pool(name="s1in", bufs=1) as inp,
            tc.tile_pool(name="s1scr", bufs=2) as scrp,
            tc.tile_pool(name="s1st", bufs=2) as stp,
            tc.tile_pool(name="evict", bufs=3) as evp,
            tc.tile_pool(name="psum", bufs=8, space="PSUM") as psum,
        ):
            ident = constp.tile([128, 128], dt.float32)
            nc.sync.dma_start(out=ident[:], in_=id_d[:])
            sm_sb = constp.tile([128, 2 * KS + IS], dt.float32)
            nc.sync.dma_start(out=sm_sb[:], in_=sm_d[:])
            g_sb = sm_sb[:, 0:KS]
            be_sb = sm_sb[:, KS:2 * KS]
            b1_sb = sm_sb[:, 2 * KS:2 * KS + IS]
            ones = constp.tile([1, 128], dt.bfloat16)
            nc.sync.dma_start(out=ones[:], in_=ones_d[:])
            b2_sb = constp.tile([1, H], dt.bfloat16)
            nc.sync.dma_start(out=b2_sb[:], in_=b2_d[:])

            lnT = lnTp.tile([128, KS, TC], dt.bfloat16)
            interT = interTp.tile([128, IS, TC], i_dt)

            for c in range(NCH):
                # ---- Stage 1: LN + transpose into lnT ----
                for tt in range(TT):
                    tr = slice(c * TC + tt * 128, c * TC + (tt + 1) * 128)
                    res = inp.tile([128, H], dt.float32, name="res")
                    nc.sync.dma_start(out=res[:], in_=res_d[tr, :])
                    s1 = stp.tile([128, 1], dt.float32, name="s1")
                    nc.vector.tensor_reduce(s1[:], res[:], mybir.AxisListType.X,
                                            ALU.add)
                    s2 = stp.tile([128, 1], dt.float32, name="s2")
                    for s in range(NS):
                        cs = slice(s * SW, (s + 1) * SW)
                        scr = scrp.tile([128, SW], dt.float32, name="scr")
                        s2p = stp.tile([128, 1], dt.float32, name="s2p")
                        nc.scalar.activation(scr[:], res[:, cs], AF.Square,
                                             accum_out=s2p[:])
                        if s == 0:
                            nc.vector.tensor_copy(s2[:], s2p[:])
                        else:
                            nc.vector.tensor_add(s2[:], s2[:], s2p[:])
                    mu = stp.tile([128, 1], dt.float32, name="mu")
                    nc.vector.tensor_scalar_mul(mu[:], s1[:], 1.0 / H)
                    mu2 = stp.tile([128, 1], dt.float32, name="mu2")
                    nc.vector.tensor_mul(mu2[:], mu[:], mu[:])
                    var = stp.tile([128, 1], dt.float32, name="var")
                    nc.vector.tensor_scalar(var[:], s2[:], 1.0 / H, float(EPS),
                                            ALU.mult, ALU.add)
                    nc.vector.tensor_sub(var[:], var[:], mu2[:])
                    sd = stp.tile([128, 1], dt.float32, name="sd")
                    nc.scalar.activation(sd[:], var[:], AF.Sqrt)
                    rstd = stp.tile([128, 1], dt.float32, name="rstd")
                    nc.vector.reciprocal(rstd[:], sd[:])
                    nmr = stp.tile([128, 1], dt.float32, name="nmr")
                    nc.vector.tensor_mul(nmr[:], mu[:], rstd[:])
                    nc.vector.tensor_scalar_mul(nmr[:], nmr[:], -1.0)

                    for s in range(NS):
                        cs = slice(s * SW, (s + 1) * SW)
                        lnp = scrp.tile([128, SW], dt.float32, name="scr")
                        nc.scalar.activation(lnp[:], res[:, cs], AF.Identity,
                                             bias=nmr[:], scale=rstd[:])
                        nq = SW // 512 if SW >= 512 else 1
                        qw = min(512, SW)
                        for q in range(nq):
                            nj = qw // 128
                            pt = psum.tile([128, qw], dt.float32, name="ps")
                            for j in range(nj):
                                nc.tensor.transpose(
                                    pt[:, j * 128:(j + 1) * 128],
                                    lnp[:, q * qw + j * 128:
                                        q * qw + (j + 1) * 128],
                                    ident[:])
                            for j in range(nj):
                                k = (s * SW + q * qw) // 128 + j
                                nc.vector.tensor_scalar(
                                    lnT[:, k, tt * 128:(tt + 1) * 128],
                                    pt[:, j * 128:(j + 1) * 128],
                                    g_sb[:, k:k + 1], be_sb[:, k:k + 1],
                                    ALU.mult, ALU.add)

                # ---- GEMM1: weight-stationary, psum[i, TCtok] over h ----
                for ib in range(IBN):
                    pA = [psum.tile([128, TC], dt.float32, name="ps")
                          for _ in range(IBS)]
                    for k in range(KS):
                        w1t = w1p.tile([128, IBW], dt.bfloat16, name="w1t")
                        nc.sync.dma_start(
                            out=w1t[:],
                            in_=w1_d[k * 128:(k + 1) * 128,
                                     ib * IBW:(ib + 1) * IBW])
                        for j in range(IBS):
                            nc.tensor.matmul(
                                pA[j][:], w1t[:, j * 128:(j + 1) * 128],
                                lnT[:, k, :],
                                start=(k == 0), stop=(k == KS - 1))
                    for j in range(IBS):
                        isl = ib * IBS + j
                        nc.scalar.activation(
                            interT[:, isl, :], pA[j][:], gelu_af,
                            bias=b1_sb[:, isl:isl + 1])

                # ---- GEMM2: token-stationary, psum[t, HCW] over i ----
                for hc in range(HCN):
                    hs = slice(hc * HCW, (hc + 1) * HCW)
                    pB = [psum.tile([128, HCW], dt.float32, name="ps")
                          for _ in range(TT)]
                    if fp8_g2:
                        for m in range(IS // 2):
                            w2t = w2p.tile([128, 2, HCW], dt.float8e4,
                                           name="w2t")
                            nc.scalar.dma_start(out=w2t[:],
                                                in_=w2_d[m, :, :, hs])
                            for tt in range(TT):
                                nc.tensor.matmul(
                                    pB[tt][:],
                                    interT[:, 2 * m:2 * m + 2,
                                           tt * 128:(tt + 1) * 128],
                                    w2t[:], start=(m == 0), stop=False,
                                    perf_mode=mybir.MatmulPerfMode.DoubleRow)
                    else:
                        for i in range(IS):
                            w2t = w2p.tile([128, HCW], dt.bfloat16, name="w2t")
                            nc.scalar.dma_start(
                                out=w2t[:],
                                in_=w2_d[i * 128:(i + 1) * 128, hs])
                            for tt in range(TT):
                                nc.tensor.matmul(
                                    pB[tt][:],
                                    interT[:, i, tt * 128:(tt + 1) * 128],
                                    w2t[:], start=(i == 0), stop=False)
                    for tt in range(TT):
                        nc.tensor.matmul(pB[tt][:], ones[:], b2_sb[:, hs],
                                         start=False, stop=True)
                    for tt in range(TT):
                        tr = slice(c * TC + tt * 128, c * TC + (tt + 1) * 128)
                        rt = evp.tile([128, HCW], dt.float32, name="rt")
                        nc.sync.dma_start(out=rt[:], in_=res_d[tr, hs])
                        if fp8_g2:
                            st = evp.tile([128, HCW], dt.float32, name="st")
                            nc.scalar.activation(st[:], pB[tt][:], AF.Identity,
                                                 scale=1.0 / W2_SCALE)
                            nc.vector.tensor_add(rt[:], st[:], rt[:])
                        else:
                            nc.vector.tensor_add(rt[:], pB[tt][:], rt[:])
                        nc.sync.dma_start(out=o_d[tr, hs], in_=rt[:])
    nc.compile()
    return nc


_NC_CACHE = None
_last_maps = None


def _get_nc():
    global _NC_CACHE
    if _NC_CACHE is None:
        _NC_CACHE = _build_nc(HIDDEN, INTER, T, TC)
    return _NC_CACHE


def _make_maps(input, residual, bias, attn_nw, attn_nb, inter_w, inter_b,
               output_w, output_b):
    H, I = HIDDEN, INTER
    KS, IS = H // 128, I // 128
    res = (np.asarray(input, np.float32).reshape(TOK, H) +
           np.asarray(residual, np.float32).reshape(TOK, H) +
           np.asarray(bias, np.float32)[None, :])
    gamma_t = np.asarray(attn_nw, np.float32).reshape(KS, 128).T
    beta_t = np.asarray(attn_nb, np.float32).reshape(KS, 128).T
    b1_t = np.asarray(inter_b, np.float32).reshape(IS, 128).T
    sm = np.ascontiguousarray(
        np.concatenate([gamma_t, beta_t, b1_t], axis=1))
    w1 = np.ascontiguousarray(
        np.asarray(inter_w, np.float32).astype(ml_dtypes.bfloat16))
    w2 = np.ascontiguousarray(
        np.asarray(output_w, np.float32).astype(ml_dtypes.bfloat16))
    b2 = np.ascontiguousarray(
        np.asarray(output_b, np.float32)[None, :].astype(ml_dtypes.bfloat16))
    ones = np.ones((1, 128), ml_dtypes.bfloat16)
    ident = np.eye(128, dtype=np.float32)

    maps = []
    for c in range(N_CORES):
        sl = slice(c * T, (c + 1) * T)
        maps.append({
            'res': np.ascontiguousarray(res[sl]),
            'sm': sm, 'w1': w1, 'w2': w2, 'b2': b2,
            'ones': ones, 'ident': ident,
        })
    return maps


def kernel(input, residual, residual_norm, bias, attn_nw, attn_nb,
           inter_w, inter_b, output_w, output_b, **kwargs):
    global _last_maps
    nc = _get_nc()
    maps = _make_maps(input, residual, bias, attn_nw, attn_nb,
                      inter_w, inter_b, output_w, output_b)
    _last_maps = maps
    res = run_bass_kernel_spmd(nc, maps, list(range(N_CORES)))
    out = np.concatenate([res.results[c]['out'] for c in range(N_CORES)], 0)
    return out.reshape(B, S, HIDDEN).astype(np.float32)


# revision 16
# speedup vs baseline: 1.4081x; 1.1196x over previous
"""DeepSpeedMLP (pre-LN fp32 path) on 8 Trainium2 NeuronCores.

Sharding: data-parallel over tokens (8192 tokens -> 1024/core); every core
streams the full inter_w/output_w (bf16) from HBM once per 512-token chunk.

Per-core pipeline, per 512-token chunk (2 chunks):
  stage 1: res (precomputed x+residual+bias on host) -> LN stats; LN apply;
           128x128 PE transposes -> lnT[h, t] bf16 (gamma/beta fused into the
           PSUM->SBUF eviction).
  GEMM1 (weight-stationary): psum[i, 512t] += w1[hslab, islab].T @ lnT[hslab]
           accumulated over all 32 h-slabs; eviction = gelu(psum + b1) -> bf16
           interT[i, t] held SBUF-resident (no transposes needed).
  GEMM2 (token-stationary): psum[t, 512h] += interT[islab, ttile].T @
           w2[islab, hchunk] accumulated over all 128 i-slabs; output_b added
           via a K=1 ones-matmul; eviction = psum + res tile -> out (single
           store, no DRAM accumulation round-trip).
"""
import sys
if '/opt/trn_rl_repo' not in sys.path:
    sys.path.insert(0, '/opt/trn_rl_repo')

import numpy as np
import ml_dtypes
import concourse.bass as bass
import concourse.mybir as mybir
import concourse.tile as tile
from concourse import bacc
from concourse.bass_utils import run_bass_kernel_spmd

dt = mybir.dt
AF = mybir.ActivationFunctionType
ALU = mybir.AluOpType

N_CORES = 8
B, S, HIDDEN, INTER = 4, 2048, 4096, 16384
TOK = B * S
T = TOK // N_CORES       # tokens per core
TC = 512                 # tokens per chunk
EPS = 1e-5
FP8_GEMM2 = True         # fp8e4 DoubleRow second GEMM (2x PE rate)
W2_SCALE = 128.0         # fp8 storage scale for output_w (std 1/128 -> ~1)


def _build_nc(H, I, T, TC, gelu_af=AF.Gelu_apprx_tanh, fp8_g2=FP8_GEMM2):
    KS = H // 128            # h-slabs
    IS = I // 128            # i-slabs
    NCH = T // TC            # token chunks
    TT = TC // 128           # t-tiles per chunk
    HCW = min(512, H)        # h-chunk width (GEMM2 psum N)
    HCN = H // HCW
    IBW = min(512, I)        # i-block width (GEMM1 psum group)
    IBN = I // IBW
    IBS = IBW // 128         # i-slabs per i-block
    SW = min(H, 1024)        # stage-1 strip width
    NS = H // SW

    nc = bacc.Bacc(None, target_bir_lowering=False)
    P = nc.declare_dram_parameter
    res_d = P("res", [T, H], dt.float32, isOutput=False)
    sm_d = P("sm", [128, 2 * KS + IS], dt.float32, isOutput=False)
    w1_d = P("w1", [H, I], dt.bfloat16, isOutput=False)
    if fp8_g2:
        w2_d = P("w2", [I // 256, 128, 2, H], dt.float8e4, isOutput=False)
        i_dt = dt.float8e4
    else:
        w2_d = P("w2", [I, H], dt.bfloat16, isOutput=False)
        i_dt = dt.bfloat16
    b2_d = P("b2", [1, H], dt.bfloat16, isOutput=False)
    ones_d = P("ones", [1, 128], dt.bfloat16, isOutput=False)
    id_d = P("ident", [128, 128], dt.float32, isOutput=False)
    o_d = P("out", [T, H], dt.float32, isOutput=True)

    with tile.TileContext(nc) as tc:
        with (
            tc.tile_pool(name="const", bufs=1) as constp,
            tc.tile_pool(name="lnT", bufs=1) as lnTp,
            tc.tile_pool(name="interT", bufs=1) as interTp,
            tc.tile_pool(name="w1p", bufs=3) as w1p,
            tc.tile_pool(name="w2p", bufs=3) as w2p,
            tc.tile_pool(name="s1in", bufs=1) as inp,
            tc.tile_pool(name="s1scr", bufs=2) as scrp,
            tc.tile_pool(name="s1st", bufs=2) as stp,
            tc.tile_pool(name="evict", bufs=3) as evp,
            tc.tile_pool(name="psum", bufs=8, space="PSUM") as psum,
        ):
            ident = constp.tile([128, 128], dt.float32)
            nc.sync.dma_start(out=ident[:], in_=id_d[:])
            sm_sb = constp.tile([128, 2 * KS + IS], dt.float32)
            nc.sync.dma_start(out=sm_sb[:], in_=sm_d[:])
            g_sb = sm_sb[:, 0:KS]
            be_sb = sm_sb[:, KS:2 * KS]
            b1_sb = sm_sb[:, 2 * KS:2 * KS + IS]
            ones = constp.tile([1, 128], dt.bfloat16)
            nc.sync.dma_start(out=ones[:], in_=ones_d[:])
            b2_sb = constp.tile([1, H], dt.bfloat16)
            nc.sync.dma_start(out=b2_sb[:], in_=b2_d[:])

            lnT = lnTp.tile([128, KS, TC], dt.bfloat16)
            interT = interTp.tile([128, IS, TC], i_dt)

            for c in range(NCH):
                # ---- Stage 1: LN + transpose into lnT ----
                for tt in range(TT):
                    tr = slice(c * TC + tt * 128, c * TC + (tt + 1) * 128)
                    res = inp.tile([128, H], dt.float32, name="res")
                    nc.sync.dma_start(out=res[:], in_=res_d[tr, :])
                    s1 = stp.tile([128, 1], dt.float32, name="s1")
                    nc.vector.tensor_reduce(s1[:], res[:], mybir.AxisListType.X,
                                            ALU.add)
                    s2 = stp.tile([128, 1], dt.float32, name="s2")
                    for s in range(NS):
                        cs = slice(s * SW, (s + 1) * SW)
                        scr = scrp.tile([128, SW], dt.float32, name="scr")
                        s2p = stp.tile([128, 1], dt.float32, name="s2p")
                        nc.scalar.activation(scr[:], res[:, cs], AF.Square,
                                             accum_out=s2p[:])
                        if s == 0:
                            nc.vector.tensor_copy(s2[:], s2p[:])
                        else:
                            nc.vector.tensor_add(s2[:], s2[:], s2p[:])
                    mu = stp.tile([128, 1], dt.float32, name="mu")
                    nc.vector.tensor_scalar_mul(mu[:], s1[:], 1.0 / H)
                    mu2 = stp.tile([128, 1], dt.float32, name="mu2")
                    nc.vector.tensor_mul(mu2[:], mu[:], mu[:])
                    var = stp.tile([128, 1], dt.float32, name="var")
                    nc.vector.tensor_scalar(var[:], s2[:], 1.0 / H, float(EPS),
                                            ALU.mult, ALU.add)
                    nc.vector.tensor_sub(var[:], var[:], mu2[:])
                    sd = stp.tile([128, 1], dt.float32, name="sd")
                    nc.scalar.activation(sd[:], var[:], AF.Sqrt)
                    rstd = stp.tile([128, 1], dt.float32, name="rstd")
                    nc.vector.reciprocal(rstd[:], sd[:])
                    nmr = stp.tile([128, 1], dt.float32, name="nmr")
                    nc.vector.tensor_mul(nmr[:], mu[:], rstd[:])
                    nc.vector.tensor_scalar_mul(nmr[:], nmr[:], -1.0)

                    for s in range(NS):
                        cs = slice(s * SW, (s + 1) * SW)
                        lnp = scrp.tile([128, SW], dt.float32, name="scr")
                        nc.scalar.activation(lnp[:], res[:, cs], AF.Identity,
                                             bias=nmr[:], scale=rstd[:])
                        nq = SW // 512 if SW >= 512 else 1
                        qw = min(512, SW)
                        for q in range(nq):
                            nj = qw // 128
                            pt = psum.tile([128, qw], dt.float32, name="ps")
                            for j in range(nj):
                                nc.tensor.transpose(
                                    pt[:, j * 128:(j + 1) * 128],
                                    lnp[:, q * qw + j * 128:
                                        q * qw + (j + 1) * 128],
                                    ident[:])
                            for j in range(nj):
                                k = (s * SW + q * qw) // 128 + j
                                nc.vector.tensor_scalar(
                                    lnT[:, k, tt * 128:(tt + 1) * 128],
                                    pt[:, j * 128:(j + 1) * 128],
                                    g_sb[:, k:k + 1], be_sb[:, k:k + 1],
                                    ALU.mult, ALU.add)

                # ---- GEMM1: weight-stationary, psum[i, TCtok] over h ----
                for ib in range(IBN):
                    pA = [psum.tile([128, TC], dt.float32, name="ps")
                          for _ in range(IBS)]
                    for k in range(KS):
                        w1t = w1p.tile([128, IBW], dt.bfloat16, name="w1t")
                        nc.sync.dma_start(
                            out=w1t[:],
                            in_=w1_d[k * 128:(k + 1) * 128,
                                     ib * IBW:(ib + 1) * IBW])
                        for j in range(IBS):
                            nc.tensor.matmul(
                                pA[j][:], w1t[:, j * 128:(j + 1) * 128],
                                lnT[:, k, :],
                                start=(k == 0), stop=(k == KS - 1))
                    for j in range(IBS):
                        isl = ib * IBS + j
                        nc.scalar.activation(
                            interT[:, isl, :], pA[j][:], gelu_af,
                            bias=b1_sb[:, isl:isl + 1])

                # ---- GEMM2: token-stationary, psum[t, HCW] over i ----
                for hc in range(HCN):
                    hs = slice(hc * HCW, (hc + 1) * HCW)
                    pB = [psum.tile([128, HCW], dt.float32, name="ps")
                          for _ in range(TT)]
                    if fp8_g2:
                        for m in range(IS // 2):
                            w2t = w2p.tile([128, 2, HCW], dt.float8e4,
                                           name="w2t")
                            nc.scalar.dma_start(out=w2t[:],
                                                in_=w2_d[m, :, :, hs])
                            for tt in range(TT):
                                nc.tensor.matmul(
                                    pB[tt][:],
                                    interT[:, 2 * m:2 * m + 2,
                                           tt * 128:(tt + 1) * 128],
                                    w2t[:], start=(m == 0), stop=False,
                                    perf_mode=mybir.MatmulPerfMode.DoubleRow)
                    else:
                        for i in range(IS):
                            w2t = w2p.tile([128, HCW], dt.bfloat16, name="w2t")
                            nc.scalar.dma_start(
                                out=w2t[:],
                                in_=w2_d[i * 128:(i + 1) * 128, hs])
                            for tt in range(TT):
                                nc.tensor.matmul(
                                    pB[tt][:],
                                    interT[:, i, tt * 128:(tt + 1) * 128],
                                    w2t[:], start=(i == 0), stop=False)
                    for tt in range(TT):
                        nc.tensor.matmul(pB[tt][:], ones[:], b2_sb[:, hs],
                                         start=False, stop=True)
                    for tt in range(TT):
                        tr = slice(c * TC + tt * 128, c * TC + (tt + 1) * 128)
                        rt = evp.tile([128, HCW], dt.float32, name="rt")
                        nc.sync.dma_start(out=rt[:], in_=res_d[tr, hs])
                        if fp8_g2:
                            st = evp.tile([128, HCW], dt.float32, name="st")
                            nc.scalar.activation(st[:], pB[tt][:], AF.Identity,
                                                 scale=1.0 / W2_SCALE)
                            nc.vector.tensor_add(rt[:], st[:], rt[:])
                        else:
                            nc.vector.tensor_add(rt[:], pB[tt][:], rt[:])
                        nc.sync.dma_start(out=o_d[tr, hs], in_=rt[:])
    nc.compile()
    return nc


_NC_CACHE = None
_last_maps = None


def _get_nc():
    global _NC_CACHE
    if _NC_CACHE is None:
        _NC_CACHE = _build_nc(HIDDEN, INTER, T, TC)
    return _NC_CACHE


def _make_maps(input, residual, bias, attn_nw, attn_nb, inter_w, inter_b,
               output_w, output_b):
    H, I = HIDDEN, INTER
    KS, IS = H // 128, I // 128
    res = (np.asarray(input, np.float32).reshape(TOK, H) +
           np.asarray(residual, np.float32).reshape(TOK, H) +
           np.asarray(bias, np.float32)[None, :])
    gamma_t = np.asarray(attn_nw, np.float32).reshape(KS, 128).T
    beta_t = np.asarray(attn_nb, np.float32).reshape(KS, 128).T
    b1_t = np.asarray(inter_b, np.float32).reshape(IS, 128).T
    sm = np.ascontiguousarray(
        np.concatenate([gamma_t, beta_t, b1_t], axis=1))
    w1 = np.ascontiguousarray(
        np.asarray(inter_w, np.float32).astype(ml_dtypes.bfloat16))
    if FP8_GEMM2:
        w2f = np.asarray(output_w, np.float32) * W2_SCALE
        w2 = np.ascontiguousarray(
            w2f.reshape(I // 256, 2, 128, H).transpose(0, 2, 1, 3)
            .astype(ml_dtypes.float8_e4m3fn))
        b2 = np.ascontiguousarray(
            (np.asarray(output_b, np.float32) * W2_SCALE)[None, :]
            .astype(ml_dtypes.bfloat16))
    else:
        w2 = np.ascontiguousarray(
            np.asarray(output_w, np.float32).astype(ml_dtypes.bfloat16))
        b2 = np.ascontiguousarray(
            np.asarray(output_b, np.float32)[None, :].astype(ml_dtypes.bfloat16))
    ones = np.ones((1, 128), ml_dtypes.bfloat16)
    ident = np.eye(128, dtype=np.float32)

    maps = []
    for c in range(N_CORES):
        sl = slice(c * T, (c + 1) * T)
        maps.append({
            'res': np.ascontiguousarray(res[sl]),
            'sm': sm, 'w1': w1, 'w2': w2, 'b2': b2,
            'ones': ones, 'ident': ident,
        })
    return maps


def kernel(input, residual, residual_norm, bias, attn_nw, attn_nb,
           inter_w, inter_b, output_w, output_b, **kwargs):
    global _last_maps
    nc = _get_nc()
    maps = _make_maps(input, residual, bias, attn_nw, attn_nb,
                      inter_w, inter_b, output_w, output_b)
    _last_maps = maps
    res = run_bass_kernel_spmd(nc, maps, list(range(N_CORES)))
    out = np.concatenate([res.results[c]['out'] for c in range(N_CORES)], 0)
    return out.reshape(B, S, HIDDEN).astype(np.float32)
